# revision 30
# baseline (speedup 1.0000x reference)
"""Type-2 NUFFT (image -> non-uniform k-space) on 8 Trainium2 NeuronCores.

kspace[b,m] = sum_{x,y} image[b,x,y] * exp(-i*(kx_m*(x-128) + ky_m*(y-128)))

Gridding (NUFFT) formulation: with an exponential-of-semicircle kernel psi
(width J, oversampled grid S > N),

  exp(-i*k*xt) ~= (1/D(xt)) * sum_g psi(k*S/2pi - g) * exp(-i*2pi*g*xt/S)

so   kspace[b,m] ~= sum_{JxJ window} F[b,g1,g2] * w1[m]*w2[m]
with F = dense DFT of the deapodized image on the S x S grid.

Work split:
  device: the dense DFT (all the heavy FLOPs), as two matmul passes
     A[g1,y] = sum_x imgd[x,y] e1[g1,x]     (stage A, complex via 2 blocks)
     F[g1,g2] = sum_y A[g1,y] e2[g2,y]      (stage B)
   sharded over 8 cores: core = (batch, g1-half of the Hermitian-half
   range [0, S/2], g2-half).  Each core outputs F f32 [97, 2*(S/2)].
  host: deapodization, trig tables, and the O(M*J^2) window interpolation
   (including Hermitian reconstruction of negative g1 rows).
"""

import sys

if '/opt/trn_rl_repo' not in sys.path:
    sys.path.insert(0, '/opt/trn_rl_repo')

import numpy as np
import ml_dtypes

B, NX, NY, M, NCORES = 2, 256, 256, 16384, 8

S = 384                  # oversampled grid (sigma = 1.5)
J = 6                    # interp kernel width (host side only)
G1H = S // 2 + 1         # Hermitian half rows: 193
RQ = 97                  # g1 rows per core (2*97 = 194 >= 193; tail padded)
SH = S // 2              # g2 cols per core: 192
BETA = np.pi * (J / 2.0) * (2.0 - 256.0 / S)

# blob layout (bf16 cols):
#   [img_x0(256) | atab_x0(2*RQ) | img_x1(256) | atab_x1(2*RQ) |
#    btab_y0(2*SH) | btab_y1(2*SH)]        btab = [Cy | Sy] (g2-half)
IMG0, ATAB0 = 0, 256
IMG1 = 256 + 2 * RQ
ATAB1 = IMG1 + 256
BTAB0 = ATAB1 + 2 * RQ
BTAB1 = BTAB0 + 3 * SH
BLOB_COLS = BTAB1 + 3 * SH

_CACHE = {}


def _es_kernel(z):
    c = J / 2.0
    out = np.zeros_like(z)
    m = np.abs(z) < c
    out[m] = np.exp(BETA * (np.sqrt(1.0 - (z[m] / c) ** 2) - 1.0))
    return out


def _deapod():
    """D(xt) = continuous FT of psi at xt/S cycles (trapezoid quadrature)."""
    c = J / 2.0
    xt = (np.arange(NX) - NX // 2).astype(np.float64)
    zq = np.linspace(-c, c, 4001)
    pz = _es_kernel(zq)
    D = np.trapezoid(pz[None, :] * np.exp(1j * 2 * np.pi * zq[None, :]
                                          * xt[:, None] / S), zq, axis=1).real
    return D


_DEAPOD = _deapod()                               # [256]
_XT = (np.arange(NX) - NX // 2).astype(np.float64)
_G2F = ((np.arange(S) + S // 2) % S - S // 2)     # col h -> g2 freq


def _tables():
    """Static device trig tables (bf16): per-half A tables + per-half B."""
    bf = ml_dtypes.bfloat16
    atabs = []
    for q in range(2):
        g = np.minimum(np.arange(q * RQ, (q + 1) * RQ), G1H - 1)
        ph = 2 * np.pi * g[None, :] * _XT[:, None] / S        # [256, RQ]
        atabs.append(np.concatenate([np.cos(ph), -np.sin(ph)],
                                    axis=1).astype(bf))       # [256, 2*RQ]
    btabs = []
    for q in range(2):
        g2 = _G2F[q * SH:(q + 1) * SH]
        ph2 = 2 * np.pi * g2[None, :] * _XT[:, None] / S      # [256, SH]
        sy = np.sin(ph2)
        btabs.append(np.concatenate([np.cos(ph2), sy, -sy],
                                    axis=1).astype(bf))       # [256, 3*SH]
    return atabs, btabs


_ATABS, _BTABS = _tables()


def _build(warm=4, aev_split=False):
    import concourse.bacc as bacc
    import concourse.mybir as mybir
    from concourse.tile import TileContext

    f32 = mybir.dt.float32
    bf16 = mybir.dt.bfloat16
    A = mybir.AluOpType

    nc = bacc.Bacc("TRN2", target_bir_lowering=False, debug=False)

    blob = nc.dram_tensor("blob", [128, BLOB_COLS], bf16, kind="ExternalInput")
    out = nc.dram_tensor("out", [128, 2 * SH], bf16, kind="ExternalOutput")

    TW = 2 * RQ      # A cols: [Ar | Ai]
    with TileContext(nc) as tc:
        with tc.tile_pool(name="const", bufs=1) as cpool, \
             tc.tile_pool(name="ps", bufs=1, space="PSUM") as ps:
            wpool = cpool

            bsb = cpool.tile([128, BLOB_COLS], bf16, name="blob")
            fsb0 = cpool.tile([128, 2 * SH], bf16, name="fsb")

            # DMA chunks in consumption order (A inputs, then both btabs --
            # fewer copies beat finer pipelining: each copy re-pays the
            # HWDGE slot + descriptor-gen latency)
            bounds = [0, BTAB0, BLOB_COLS]
            for i in range(len(bounds) - 1):
                cs = slice(bounds[i], bounds[i + 1])
                nc.sync.dma_start(bsb[:, cs], blob[:, cs])

            if warm:
                # p-state warm-up: dummy matmuls reading fsb before it is
                # written (values irrelevant, wab is never read)
                wab = ps.tile([128, 384], f32, tag="warm", bufs=1)
                for _ in range(warm):
                    nc.tensor.matmul(wab[:, :], fsb0[:, 0:128],
                                     fsb0[:, 0:384], start=True,
                                     stop=True)

            # stage A: A^T[y, (Ar|Ai)] per y-chunk, contract x (2 chunks);
            # evict adds a negated-Ar block: asb = [Ar | Ai | -Ar]
            asb = []
            for yc in range(2):
                aps = ps.tile([128, TW], f32, tag="aps", bufs=2)
                for xc in range(2):
                    imgc = IMG0 if xc == 0 else IMG1
                    atc = ATAB0 if xc == 0 else ATAB1
                    nc.tensor.matmul(aps[:, :],
                                     bsb[:, imgc + yc * 128:imgc + yc * 128 + 128],
                                     bsb[:, atc:atc + TW],
                                     start=(xc == 0), stop=(xc == 1))
                a = wpool.tile([128, TW], bf16, tag=f"asb{yc}")
                if aev_split:
                    nc.vector.tensor_scalar(a[:, 0:RQ], aps[:, 0:RQ],
                                            scalar1=1.0, scalar2=0.0,
                                            op0=A.mult, op1=A.add)
                    nc.scalar.copy(a[:, RQ:TW], aps[:, RQ:TW])
                elif yc == 0:
                    nc.vector.tensor_scalar(a[:, :], aps[:, :], scalar1=1.0,
                                            scalar2=0.0, op0=A.mult,
                                            op1=A.add)
                else:
                    nc.scalar.copy(a[:, :], aps[:, :])
                asb.append(a)

            # stage B: Fr/Fi (separate PSUM banks); btab_yc = [Cy | Sy]
            fr = ps.tile([128, SH], f32, tag="fr", bufs=1)
            fi = ps.tile([128, SH], f32, tag="fi", bufs=1)
            for yc in range(2):
                bt = BTAB0 if yc == 0 else BTAB1
                ar = asb[yc][:, 0:RQ]
                ai = asb[yc][:, RQ:TW]
                st, sp = (yc == 0), (yc == 1)
                # Fr = Ar*Cy + Ai*Sy ; Fi = Ai*Cy + Ar*(-Sy)
                nc.tensor.matmul(fr[0:RQ, :], ar, bsb[:, bt:bt + SH],
                                 start=st, stop=False)
                nc.tensor.matmul(fr[0:RQ, :], ai, bsb[:, bt + SH:bt + 2 * SH],
                                 start=False, stop=sp)
                nc.tensor.matmul(fi[0:RQ, :], ai, bsb[:, bt:bt + SH],
                                 start=st, stop=False)
                nc.tensor.matmul(fi[0:RQ, :], ar, bsb[:, bt + 2 * SH:bt + 3 * SH],
                                 start=False, stop=sp)

            # evict F (f32) on DVE/Act in parallel, then one combined
            # out-DMA from SP (two copies would serialize on HWDGE)
            fsb = fsb0
            nc.vector.tensor_scalar(fsb[0:RQ, 0:SH], fr[0:RQ, :], scalar1=1.0,
                                    scalar2=0.0, op0=A.mult, op1=A.add)
            nc.scalar.copy(fsb[0:RQ, SH:2 * SH], fi[0:RQ, :])
            nc.sync.dma_start(out[0:RQ, :], fsb[0:RQ, :])

    nc.compile()
    return nc


def _host_prep(image, trajectory):
    bf = ml_dtypes.bfloat16
    imgd = (image / (_DEAPOD[None, :, None] * _DEAPOD[None, None, :])
            ).astype(bf)                                   # [B, 256, 256]
    blobs = np.zeros((NCORES, 128, BLOB_COLS), dtype=bf)
    for c in range(NCORES):
        b, q1, q2 = c // 4, (c // 2) % 2, c % 2
        blobs[c, :, IMG0:IMG0 + 256] = imgd[b, 0:128, :]
        blobs[c, :, IMG1:IMG1 + 256] = imgd[b, 128:256, :]
        blobs[c, :, ATAB0:ATAB0 + 2 * RQ] = _ATABS[q1][0:128]
        blobs[c, :, ATAB1:ATAB1 + 2 * RQ] = _ATABS[q1][128:256]
        blobs[c, :, BTAB0:BTAB0 + 3 * SH] = _BTABS[q2][0:128]
        blobs[c, :, BTAB1:BTAB1 + 3 * SH] = _BTABS[q2][128:256]
    return blobs


def _host_interp(F, trajectory):
    """F: [B, G1H, S] complex64 (g1 = 0..S/2; col h -> freq _G2F[h])."""
    kx = trajectory[0].astype(np.float64)
    ky = trajectory[1].astype(np.float64)
    eta1 = kx * S / (2 * np.pi)
    eta2 = ky * S / (2 * np.pi)
    a0 = np.floor(eta1).astype(int) - J // 2 + 1
    b0 = np.floor(eta2).astype(int) - J // 2 + 1
    js = np.arange(J)
    w1 = _es_kernel(eta1[:, None] - (a0[:, None] + js[None, :]))  # [M, J]
    w2 = _es_kernel(eta2[:, None] - (b0[:, None] + js[None, :]))
    gg1 = (a0[:, None] + js[None, :]) % S                         # [M, J]
    gg2 = (b0[:, None] + js[None, :]) % S

    # full F grid [B, S, S] indexed by (g1 mod S, g2 mod S)
    Ffull = np.zeros((B, S, S), dtype=np.complex64)
    q2 = (_G2F % S)
    rows = np.arange(G1H)
    Ffull[:, rows[:, None] % S, q2[None, :]] = F
    neg = np.arange(1, S // 2)
    mirr = (S - np.arange(S)) % S
    Ffull[:, (-neg) % S, :] = np.conj(Ffull[:, neg][:, :, mirr])

    vals = Ffull[:, gg1[:, :, None], gg2[:, None, :]]             # [B, M, J, J]
    w = (w1[:, :, None] * w2[:, None, :]).astype(np.float32)      # [M, J, J]
    return (vals * w[None]).sum(axis=(2, 3)).astype(np.complex64)


def kernel(image, trajectory):
    from concourse.bass_utils import run_bass_kernel_spmd

    if 'nc' not in _CACHE:
        _CACHE['nc'] = _build()
    nc = _CACHE['nc']

    image = np.ascontiguousarray(np.asarray(image, dtype=np.float32))
    trajectory = np.ascontiguousarray(np.asarray(trajectory, dtype=np.float32))
    blobs = _host_prep(image, trajectory)

    in_maps = [{"blob": np.ascontiguousarray(blobs[c])} for c in range(NCORES)]
    res = run_bass_kernel_spmd(nc, in_maps, core_ids=list(range(NCORES)))

    F = np.zeros((B, G1H, S), dtype=np.complex64)
    for c in range(NCORES):
        b, q1, q2 = c // 4, (c // 2) % 2, c % 2
        o = res.results[c]["out"].astype(np.float32)   # [128, 2*SH]
        lo = q1 * RQ
        hi = min(lo + RQ, G1H)
        F[b, lo:hi, q2 * SH:(q2 + 1) * SH] = (o[0:hi - lo, 0:SH]
                                              + 1j * o[0:hi - lo, SH:2 * SH])

    return _host_interp(F, trajectory)


# revision 33
# speedup vs baseline: 1.0034x; 1.0034x over previous
"""Type-2 NUFFT (image -> non-uniform k-space) on 8 Trainium2 NeuronCores.

kspace[b,m] = sum_{x,y} image[b,x,y] * exp(-i*(kx_m*(x-128) + ky_m*(y-128)))

Gridding (NUFFT) formulation: with an exponential-of-semicircle kernel psi
(width J, oversampled grid S > N),

  exp(-i*k*xt) ~= (1/D(xt)) * sum_g psi(k*S/2pi - g) * exp(-i*2pi*g*xt/S)

so   kspace[b,m] ~= sum_{JxJ window} F[b,g1,g2] * w1[m]*w2[m]
with F = dense DFT of the deapodized image on the S x S grid.

Work split:
  device: the dense DFT (all the heavy FLOPs), as two matmul passes
     A[g1,y] = sum_x imgd[x,y] e1[g1,x]     (stage A, complex via 2 blocks)
     F[g1,g2] = sum_y A[g1,y] e2[g2,y]      (stage B)
   sharded over 8 cores: core = (batch, g1-half of the Hermitian-half
   range [0, S/2], g2-half).  Each core outputs F f32 [97, 2*(S/2)].
  host: deapodization, trig tables, and the O(M*J^2) window interpolation
   (including Hermitian reconstruction of negative g1 rows).
"""

import sys

if '/opt/trn_rl_repo' not in sys.path:
    sys.path.insert(0, '/opt/trn_rl_repo')

import numpy as np
import ml_dtypes

B, NX, NY, M, NCORES = 2, 256, 256, 16384, 8

S = 384                  # oversampled grid (sigma = 1.5)
J = 6                    # interp kernel width (host side only)
G1H = S // 2 + 1         # Hermitian half rows: 193
RQ = 97                  # g1 rows per core (2*97 = 194 >= 193; tail padded)
SH = S // 2              # g2 cols per core: 192
BETA = np.pi * (J / 2.0) * (2.0 - 256.0 / S)

# blob layout (bf16 cols):
#   [img_x0(256) | atab_x0(2*RQ) | img_x1(256) | atab_x1(2*RQ) |
#    btab_y0(2*SH) | btab_y1(2*SH)]        btab = [Cy | Sy] (g2-half)
IMG0, ATAB0 = 0, 256
IMG1 = 256 + 2 * RQ
ATAB1 = IMG1 + 256
BTAB0 = ATAB1 + 2 * RQ
BTAB1 = BTAB0 + 3 * SH
BLOB_COLS = BTAB1 + 3 * SH

_CACHE = {}


def _es_kernel(z):
    c = J / 2.0
    out = np.zeros_like(z)
    m = np.abs(z) < c
    out[m] = np.exp(BETA * (np.sqrt(1.0 - (z[m] / c) ** 2) - 1.0))
    return out


def _deapod():
    """D(xt) = continuous FT of psi at xt/S cycles (trapezoid quadrature)."""
    c = J / 2.0
    xt = (np.arange(NX) - NX // 2).astype(np.float64)
    zq = np.linspace(-c, c, 4001)
    pz = _es_kernel(zq)
    D = np.trapezoid(pz[None, :] * np.exp(1j * 2 * np.pi * zq[None, :]
                                          * xt[:, None] / S), zq, axis=1).real
    return D


_DEAPOD = _deapod()                               # [256]
_XT = (np.arange(NX) - NX // 2).astype(np.float64)
_G2F = ((np.arange(S) + S // 2) % S - S // 2)     # col h -> g2 freq


def _tables():
    """Static device trig tables (bf16): per-half A tables + per-half B."""
    bf = ml_dtypes.bfloat16
    atabs = []
    for q in range(2):
        g = np.minimum(np.arange(q * RQ, (q + 1) * RQ), G1H - 1)
        ph = 2 * np.pi * g[None, :] * _XT[:, None] / S        # [256, RQ]
        atabs.append(np.concatenate([np.cos(ph), -np.sin(ph)],
                                    axis=1).astype(bf))       # [256, 2*RQ]
    btabs = []
    for q in range(2):
        g2 = _G2F[q * SH:(q + 1) * SH]
        ph2 = 2 * np.pi * g2[None, :] * _XT[:, None] / S      # [256, SH]
        sy = np.sin(ph2)
        btabs.append(np.concatenate([np.cos(ph2), sy, -sy],
                                    axis=1).astype(bf))       # [256, 3*SH]
    return atabs, btabs


_ATABS, _BTABS = _tables()


def _build(warm=4, aev_split=False):
    import concourse.bacc as bacc
    import concourse.mybir as mybir
    from concourse.tile import TileContext

    f32 = mybir.dt.float32
    bf16 = mybir.dt.bfloat16
    A = mybir.AluOpType

    nc = bacc.Bacc("TRN2", target_bir_lowering=False, debug=False)

    blob = nc.dram_tensor("blob", [128, BLOB_COLS], bf16, kind="ExternalInput")
    out = nc.dram_tensor("out", [128, 2 * SH], bf16, kind="ExternalOutput")

    TW = 2 * RQ      # A cols: [Ar | Ai]
    with TileContext(nc) as tc:
        with tc.tile_pool(name="const", bufs=1) as cpool, \
             tc.tile_pool(name="ps", bufs=1, space="PSUM") as ps:
            wpool = cpool

            bsb = cpool.tile([128, BLOB_COLS], bf16, name="blob")
            fsb0 = cpool.tile([128, 2 * SH], bf16, name="fsb")

            # DMA chunks in consumption order (A inputs, then both btabs --
            # fewer copies beat finer pipelining: each copy re-pays the
            # HWDGE slot + descriptor-gen latency)
            bounds = [0, BTAB0, BLOB_COLS]
            for i in range(len(bounds) - 1):
                cs = slice(bounds[i], bounds[i + 1])
                nc.sync.dma_start(bsb[:, cs], blob[:, cs])

            if warm:
                # p-state warm-up: dummy matmuls reading fsb before it is
                # written (values irrelevant, wab is never read)
                wab = ps.tile([128, 384], f32, tag="warm", bufs=1)
                for _ in range(warm):
                    nc.tensor.matmul(wab[:, :], fsb0[:, 0:128],
                                     fsb0[:, 0:384], start=True,
                                     stop=True)

            # stage A: A^T[y, (Ar|Ai)] per y-chunk, contract x (2 chunks);
            # evict adds a negated-Ar block: asb = [Ar | Ai | -Ar]
            asb = []
            for yc in range(2):
                aps = ps.tile([128, TW], f32, tag="aps", bufs=2)
                for xc in range(2):
                    imgc = IMG0 if xc == 0 else IMG1
                    atc = ATAB0 if xc == 0 else ATAB1
                    nc.tensor.matmul(aps[:, :],
                                     bsb[:, imgc + yc * 128:imgc + yc * 128 + 128],
                                     bsb[:, atc:atc + TW],
                                     start=(xc == 0), stop=(xc == 1))
                a = wpool.tile([128, TW], bf16, tag=f"asb{yc}")
                if aev_split:
                    nc.vector.tensor_scalar(a[:, 0:RQ], aps[:, 0:RQ],
                                            scalar1=1.0, scalar2=0.0,
                                            op0=A.mult, op1=A.add)
                    nc.scalar.copy(a[:, RQ:TW], aps[:, RQ:TW])
                elif yc == 0:
                    nc.vector.tensor_scalar(a[:, :], aps[:, :], scalar1=1.0,
                                            scalar2=0.0, op0=A.mult,
                                            op1=A.add)
                else:
                    nc.scalar.copy(a[:, :], aps[:, :])
                asb.append(a)

            # stage B: Fr/Fi (separate PSUM banks); btab_yc = [Cy | Sy]
            fr = ps.tile([128, SH], f32, tag="fr", bufs=1)
            fi = ps.tile([128, SH], f32, tag="fi", bufs=1)
            for yc in range(2):
                bt = BTAB0 if yc == 0 else BTAB1
                ar = asb[yc][:, 0:RQ]
                ai = asb[yc][:, RQ:TW]
                st, sp = (yc == 0), (yc == 1)
                # Fr = Ar*Cy + Ai*Sy ; Fi = Ai*Cy + Ar*(-Sy)
                nc.tensor.matmul(fr[0:RQ, :], ar, bsb[:, bt:bt + SH],
                                 start=st, stop=False)
                nc.tensor.matmul(fr[0:RQ, :], ai, bsb[:, bt + SH:bt + 2 * SH],
                                 start=False, stop=sp)
                nc.tensor.matmul(fi[0:RQ, :], ai, bsb[:, bt:bt + SH],
                                 start=st, stop=False)
                nc.tensor.matmul(fi[0:RQ, :], ar, bsb[:, bt + 2 * SH:bt + 3 * SH],
                                 start=False, stop=sp)

            # evict F (f32) on DVE/Act in parallel, then one combined
            # out-DMA from SP (two copies would serialize on HWDGE)
            fsb = fsb0
            nc.scalar.copy(fsb[0:RQ, 0:SH], fr[0:RQ, :])
            nc.vector.tensor_scalar(fsb[0:RQ, SH:2 * SH], fi[0:RQ, :],
                                    scalar1=1.0, scalar2=0.0, op0=A.mult,
                                    op1=A.add)
            nc.sync.dma_start(out[0:RQ, :], fsb[0:RQ, :])

    nc.compile()
    return nc


def _host_prep(image, trajectory):
    bf = ml_dtypes.bfloat16
    imgd = (image / (_DEAPOD[None, :, None] * _DEAPOD[None, None, :])
            ).astype(bf)                                   # [B, 256, 256]
    blobs = np.zeros((NCORES, 128, BLOB_COLS), dtype=bf)
    for c in range(NCORES):
        b, q1, q2 = c // 4, (c // 2) % 2, c % 2
        blobs[c, :, IMG0:IMG0 + 256] = imgd[b, 0:128, :]
        blobs[c, :, IMG1:IMG1 + 256] = imgd[b, 128:256, :]
        blobs[c, :, ATAB0:ATAB0 + 2 * RQ] = _ATABS[q1][0:128]
        blobs[c, :, ATAB1:ATAB1 + 2 * RQ] = _ATABS[q1][128:256]
        blobs[c, :, BTAB0:BTAB0 + 3 * SH] = _BTABS[q2][0:128]
        blobs[c, :, BTAB1:BTAB1 + 3 * SH] = _BTABS[q2][128:256]
    return blobs


def _host_interp(F, trajectory):
    """F: [B, G1H, S] complex64 (g1 = 0..S/2; col h -> freq _G2F[h])."""
    kx = trajectory[0].astype(np.float64)
    ky = trajectory[1].astype(np.float64)
    eta1 = kx * S / (2 * np.pi)
    eta2 = ky * S / (2 * np.pi)
    a0 = np.floor(eta1).astype(int) - J // 2 + 1
    b0 = np.floor(eta2).astype(int) - J // 2 + 1
    js = np.arange(J)
    w1 = _es_kernel(eta1[:, None] - (a0[:, None] + js[None, :]))  # [M, J]
    w2 = _es_kernel(eta2[:, None] - (b0[:, None] + js[None, :]))
    gg1 = (a0[:, None] + js[None, :]) % S                         # [M, J]
    gg2 = (b0[:, None] + js[None, :]) % S

    # full F grid [B, S, S] indexed by (g1 mod S, g2 mod S)
    Ffull = np.zeros((B, S, S), dtype=np.complex64)
    q2 = (_G2F % S)
    rows = np.arange(G1H)
    Ffull[:, rows[:, None] % S, q2[None, :]] = F
    neg = np.arange(1, S // 2)
    mirr = (S - np.arange(S)) % S
    Ffull[:, (-neg) % S, :] = np.conj(Ffull[:, neg][:, :, mirr])

    vals = Ffull[:, gg1[:, :, None], gg2[:, None, :]]             # [B, M, J, J]
    w = (w1[:, :, None] * w2[:, None, :]).astype(np.float32)      # [M, J, J]
    return (vals * w[None]).sum(axis=(2, 3)).astype(np.complex64)


def kernel(image, trajectory):
    from concourse.bass_utils import run_bass_kernel_spmd

    if 'nc' not in _CACHE:
        _CACHE['nc'] = _build()
    nc = _CACHE['nc']

    image = np.ascontiguousarray(np.asarray(image, dtype=np.float32))
    trajectory = np.ascontiguousarray(np.asarray(trajectory, dtype=np.float32))
    blobs = _host_prep(image, trajectory)

    in_maps = [{"blob": np.ascontiguousarray(blobs[c])} for c in range(NCORES)]
    res = run_bass_kernel_spmd(nc, in_maps, core_ids=list(range(NCORES)))

    F = np.zeros((B, G1H, S), dtype=np.complex64)
    for c in range(NCORES):
        b, q1, q2 = c // 4, (c // 2) % 2, c % 2
        o = res.results[c]["out"].astype(np.float32)   # [128, 2*SH]
        lo = q1 * RQ
        hi = min(lo + RQ, G1H)
        F[b, lo:hi, q2 * SH:(q2 + 1) * SH] = (o[0:hi - lo, 0:SH]
                                              + 1j * o[0:hi - lo, SH:2 * SH])

    return _host_interp(F, trajectory)
